# revision 27
# baseline (speedup 1.0000x reference)
"""CQCNN piece estimator on 8 trn2 NeuronCores.

Strategy: pure data parallel over batch (8192 samples/core), SPMD (one NEFF).
Activations live feature-major [features(partitions), batch(free)].
Convs on the 6x6/3x3 boards are dense linear maps -> matmuls with
zero-block skipping.  Maxpool via DMA parity-gather + 3 tensor_max.
Quantum sim runs qubit-interleaved on 128 partitions.  softmax(2)/sigmoid
are rewritten in terms of tanh so one ACT table set serves everything.
"""

import numpy as np
import ml_dtypes

import concourse.bass as bass
import concourse.bacc as bacc
import concourse.mybir as mybir
import concourse.tile as tile
from concourse.bass_utils import run_bass_kernel_spmd
import concourse.bass_utils as _bu

# hide LDWEIGHTS behind matmuls (double-buffered weight loads); the harness
# disables this walrus opt by default
_orig_run_command = _bu.run_command


def _run_command_ldwopt(argv, **kwargs):
    argv = ["--enable-ldw-opt=true" if a == "--enable-ldw-opt=false" else a
            for a in argv]
    return _orig_run_command(argv, **kwargs)


# _bu.run_command = _run_command_ldwopt  # walrus codegen rejects ldw-opt

BF16 = mybir.dt.bfloat16
F32 = mybir.dt.float32
nbf = ml_dtypes.bfloat16

B = 65536
NCORES = 8
BC = B // NCORES          # 8192 per core
CB = 2048                 # chunk of batch processed per pipeline pass
NCHUNK = BC // CB         # 4
NSL = 512                 # matmul moving-operand slice
NSLC = CB // NSL          # 4 slices per chunk
NQ = 8

AF = mybir.ActivationFunctionType
ALU = mybir.AluOpType

_cache = {}



def _build_conv_maps(conv1_w, conv2_w, conv3_w):
    """Dense linear maps for the three convs, with my feature orderings.

    X in-features  : channel-major c*36 + y*6 + x   (== board reshape order)
    H1 out-features: pos-major (y*6+x)*16 + c
    H2 out-features: pos-major (y*6+x)*32 + c
    P  (pooled)    : pos-major (y*3+x)*32 + c
    H3 out-features: pos-major (y*3+x)*64 + c
    """
    T1 = np.zeros((108, 576), np.float32)
    for co in range(16):
        for ci in range(3):
            for ky in range(3):
                for kx in range(3):
                    w = conv1_w[co, ci, ky, kx]
                    for yo in range(6):
                        yi = yo + ky - 1
                        if not 0 <= yi < 6:
                            continue
                        for xo in range(6):
                            xi = xo + kx - 1
                            if 0 <= xi < 6:
                                T1[ci * 36 + yi * 6 + xi, (yo * 6 + xo) * 16 + co] = w

    T2 = np.zeros((576, 1152), np.float32)
    for ky in range(3):
        for kx in range(3):
            w = conv2_w[:, :, ky, kx]  # [32,16]
            for yo in range(6):
                yi = yo + ky - 1
                if not 0 <= yi < 6:
                    continue
                for xo in range(6):
                    xi = xo + kx - 1
                    if 0 <= xi < 6:
                        par = (yo % 2) * 2 + (xo % 2)
                        qo = (yo // 2) * 3 + (xo // 2)
                        pi, po = (yi * 6 + xi) * 16, par * 288 + qo * 32
                        T2[pi:pi + 16, po:po + 32] = w.T

    T3 = np.zeros((288, 576), np.float32)
    for ky in range(3):
        for kx in range(3):
            w = conv3_w[:, :, ky, kx]  # [64,32]
            for yo in range(3):
                yi = yo + ky - 1
                if not 0 <= yi < 3:
                    continue
                for xo in range(3):
                    xi = xo + kx - 1
                    if 0 <= xi < 3:
                        pi, po = (yi * 3 + xi) * 32, (yo * 3 + xo) * 64
                        T3[pi:pi + 32, po:po + 64] = w.T
    return T1, T2, T3


def _parts(n, step=128):
    return [(i, min(i + step, n)) for i in range(0, n, step)]


def _mk_layout():
    k2 = _parts(576)
    k3 = _parts(288, 96)
    km = _parts(584)
    off16, c = {}, 0
    def a16(name, cols):
        nonlocal c
        off16[name] = c
        c += cols
    a16("t1", 576)
    for i in range(len(k2)): a16(f"t2_{i}", 1152)
    for i in range(len(k3)): a16(f"t3_{i}", 576)
    for i in range(len(km)): a16(f"w1_{i}", 192)
    a16("w2", 64); a16("w3", 3); a16("smat", 3)
    n16 = c
    off32, c2 = {}, 0
    def a32(name, cols):
        nonlocal c2
        off32[name] = c2
        c2 += cols
    a32("b1", 5); a32("b2", 12); a32("b3", 5); a32("bm1", 2)
    a32("bm2", 1); a32("bh", 1); a32("rot", 9)
    a32("qx", BC // 16); a32("qxn", BC // 16)
    return off16, n16, off32, c2


OFF16, NC16, OFF32, NC32 = _mk_layout()


def _nonzero_blocks(T, kparts, mparts):
    out = {}
    for mj, (m0, m1) in enumerate(mparts):
        ks = [ki for ki, (k0, k1) in enumerate(kparts)
              if np.any(T[k0:k1, m0:m1])]
        out[mj] = ks
    return out


def _build_program():
    nc = bacc.Bacc("TRN2", target_bir_lowering=False, debug=False)

    xT_d = nc.dram_tensor("xT", [108, BC], BF16, kind="ExternalInput")
    wb16_d = nc.dram_tensor("wb16", [128, NC16], BF16, kind="ExternalInput")
    wb32_d = nc.dram_tensor("wb32", [128, NC32], F32, kind="ExternalInput")
    out_d = nc.dram_tensor("out", [3, BC], F32, kind="ExternalOutput")

    k1 = _parts(108)          # 1 part
    m1p = _parts(576)         # 5
    k2 = _parts(576)          # 5
    m2p = [(p * 288 + o0, p * 288 + o1)
           for p in range(4) for o0, o1 in ((0, 96), (96, 192), (192, 288))]
    k3 = _parts(288, 96)      # 3 x 96 (one pooled row each)
    m3p = _parts(576)         # 5
    km = _parts(584)          # 5  (last 72 = conv tail 64 + quantum 8)

    blocks2 = _cache["blocks2"]
    blocks3 = _cache["blocks3"]

    from contextlib import ExitStack
    with tile.TileContext(nc) as tc, ExitStack() as ctx:
        wts = ctx.enter_context(tc.tile_pool(name="wts", bufs=1))
        qp = ctx.enter_context(tc.tile_pool(name="qp", bufs=1))
        xp = ctx.enter_context(tc.tile_pool(name="xp", bufs=2))
        h1p = ctx.enter_context(tc.tile_pool(name="h1p", bufs=1))
        prp = ctx.enter_context(tc.tile_pool(name="prp", bufs=1))
        pp = ctx.enter_context(tc.tile_pool(name="pp", bufs=2))
        h3qp = ctx.enter_context(tc.tile_pool(name="h3qp", bufs=2))
        h3p = ctx.enter_context(tc.tile_pool(name="h3p", bufs=1))
        hdp = ctx.enter_context(tc.tile_pool(name="hdp", bufs=1))
        psp = ctx.enter_context(tc.tile_pool(name="psp", bufs=4, space="PSUM"))

        # two packed weight blocks -> two DMA dispatches instead of ~25
        wb16 = wts.tile([128, NC16], BF16, tag="wb16", name="wb16")
        cuts = [0, OFF16["t2_0"], OFF16["t2_2"], OFF16["t2_4"], OFF16["t3_0"], NC16]
        for ci in range(len(cuts) - 1):
            nc.sync.dma_start(out=wb16[:, cuts[ci]:cuts[ci + 1]],
                              in_=wb16_d[:, cuts[ci]:cuts[ci + 1]])
        wb32 = wts.tile([128, NC32], F32, tag="wb32", name="wb32")
        nc.sync.dma_start(out=wb32, in_=wb32_d[:, :])

        def v16(off, rows, cols):
            return wb16[:rows, off:off + cols]

        def v32(off, rows, cols):
            return wb32[:rows, off:off + cols]

        o = dict(OFF16)
        t1 = v16(o["t1"], 108, 576)
        t2s = [v16(o[f"t2_{i}"], k1_ - k0_, 1152) for i, (k0_, k1_) in enumerate(k2)]
        t3s = [v16(o[f"t3_{i}"], k1_ - k0_, 576) for i, (k0_, k1_) in enumerate(k3)]
        w1s = [v16(o[f"w1_{i}"], k1_ - k0_, 192) for i, (k0_, k1_) in enumerate(km)]
        w2 = v16(o["w2"], 128, 64)
        w3 = v16(o["w3"], 128, 3)
        smat = v16(o["smat"], 3, 3)
        o2 = dict(OFF32)
        b1 = v32(o2["b1"], 128, 5)
        b2 = v32(o2["b2"], 128, 12)
        b3 = v32(o2["b3"], 128, 5)
        bm1 = v32(o2["bm1"], 128, 2)
        bm2 = v32(o2["bm2"], 128, 1)
        bh = v32(o2["bh"], 3, 1)
        rot = v32(o2["rot"], 128, 9)
        qx_v = v32(o2["qx"], 128, BC // 16)
        qxn_v = v32(o2["qxn"], 128, BC // 16)

        zc = wts.tile([128, 1], F32, tag="zc", name="zc")
        nc.vector.memset(zc, 0.0)
        halfpi = wts.tile([128, 1], F32, tag="halfpi", name="halfpi")
        nc.vector.memset(halfpi, float(np.pi / 2))

        # ---- quantum sim, qubit-interleaved [q + 8g, j], b = g*512 + j ----
        # (emitted after chunk 0's conv work so PE warms up while ACT/DVE
        # run the sin/cos/tanh chain)
        def emit_quantum():
            qx, qxn = qx_v, qxn_v
            qst = None
            for l in range(3):
                sa = qp.tile([128, BC // 16], F32, tag="sa", name=f"sa{l}")
                ca = qp.tile([128, BC // 16], F32, tag="ca", name=f"ca{l}")
                nc.vector.tensor_scalar_mul(sa, qx, rot[:, 3 * l:3 * l + 1])
                nc.vector.tensor_scalar_mul(ca, qxn, rot[:, 3 * l + 1:3 * l + 2])
                nc.scalar.activation(sa, sa, AF.Sin, bias=zc)
                nc.scalar.activation(ca, ca, AF.Sin, bias=halfpi)
                sc = qp.tile([128, BC // 16], F32, tag="sc", name=f"sc{l}")
                nc.vector.tensor_mul(sc, sa, ca)
                if qst is None:
                    qst = sc
                else:
                    ta = qp.tile([128, BC // 16], F32, tag="ta", name=f"ta{l}")
                    nc.vector.tensor_scalar_mul(ta, qst, rot[:, 3 * l + 2:3 * l + 3])
                    nc.scalar.activation(ta, ta, AF.Tanh, bias=zc)
                    qn = qp.tile([128, BC // 16], F32, tag="qn", name=f"qn{l}")
                    nc.vector.tensor_add(qn, sc, ta)
                    qst = qn
            qfb = qp.tile([128, BC // 16], BF16, tag="qfb", name="qfb")
            nc.vector.tensor_copy(qfb, qst)
            return qfb

        # ---- software-pipelined chunk stages ----
        # A(c): load + conv1 + conv2 (-> parity tiles) + pool
        # B(c): conv3 + mlp + heads + store
        # emission: A0, quantum, A1, B0, A2, B1, A3, B2, B3 keeps PE fed
        # through each chunk's pool boundary.
        st = {}

        def stage_a(c, qfb):
            c0 = c * CB
            xc = xp.tile([108, CB], BF16, tag="xc", name="xc")
            nc.sync.dma_start(out=xc, in_=xT_d[:, c0:c0 + CB])
            h1 = [h1p.tile([m1_ - m0_, CB], BF16, tag=f"h1_{i}", name=f"h1_{i}")
                  for i, (m0_, m1_) in enumerate(m1p)]
            par = [prp.tile([96, CB], BF16,
                            tag=f"par_{j}", name=f"par_{j}") for j in range(12)]
            pool = [pp.tile([96, CB], BF16, tag=f"pool_{t}", name=f"pool_{t}")
                    for t in range(3)]
            h3t4 = h3qp.tile([72, CB], BF16, tag="h3t4", name="h3t4")
            if qfb is not None:
                for g in range(4 * c, 4 * c + 4):
                    o = (g - 4 * c) * 512
                    nc.sync.dma_start(out=h3t4[64:72, o:o + 512],
                                      in_=qfb[g * 8:(g + 1) * 8, :])

            for mj, (m0_, m1_) in enumerate(m1p):
                r = m1_ - m0_
                for h in range(2):
                    hb = slice(h * 1024, (h + 1) * 1024)
                    ps = psp.tile([128, 1024], F32, tag="ps", name="ps")
                    for s in range(2):
                        sl = slice(h * 1024 + s * NSL, h * 1024 + (s + 1) * NSL)
                        pl = slice(s * NSL, (s + 1) * NSL)
                        nc.tensor.matmul(ps[:r, pl], t1[:, m0_:m1_], xc[:, sl],
                                         start=True, stop=True)
                    nc.scalar.activation(h1[mj][:, hb], ps[:r], AF.Relu,
                                         bias=b1[:r, mj:mj + 1])
            for mj, (m0_, m1_) in enumerate(m2p):
                r = m1_ - m0_
                ks = blocks2[mj]
                p_, t_ = mj // 3, mj % 3
                for h in range(2):
                    hb = slice(h * 1024, (h + 1) * 1024)
                    ps = psp.tile([128, 1024], F32, tag="ps", name="ps")
                    for s in range(2):
                        sl = slice(h * 1024 + s * NSL, h * 1024 + (s + 1) * NSL)
                        pl = slice(s * NSL, (s + 1) * NSL)
                        for i, ki in enumerate(ks):
                            nc.tensor.matmul(ps[:r, pl], t2s[ki][:, m0_:m1_],
                                             h1[ki][:, sl],
                                             start=(i == 0), stop=(i == len(ks) - 1))
                    if (p_ + t_) % 2 == 0:
                        nc.scalar.activation(par[mj][:, hb], ps[:r], AF.Relu,
                                             bias=b2[:r, mj:mj + 1])
                    else:
                        nc.vector.tensor_scalar(par[mj][:, hb], ps[:r],
                                                b2[:r, mj:mj + 1], 0.0,
                                                ALU.add, ALU.max)
            # maxpool across the 4 parity copies (elementwise, lane-aligned)
            for t in range(3):
                nc.vector.tensor_max(pool[t], par[t], par[3 + t])
                nc.vector.tensor_max(par[6 + t], par[6 + t], par[9 + t])
                nc.vector.tensor_max(pool[t], pool[t], par[6 + t])
            st[c] = (pool, h3t4)

        def stage_b(c):
            c0 = c * CB
            pool, h3t4 = st.pop(c)
            h3 = [h3p.tile([128, CB], BF16, tag=f"h3_{i}", name=f"h3_{i}")
                  for i in range(4)]
            amlp = hdp.tile([128, CB], BF16, tag="amlp", name="amlp")
            fmlp = hdp.tile([128, CB], BF16, tag="fmlp", name="fmlp")
            lb = hdp.tile([3, CB], BF16, tag="lb", name="lb")
            ob = hdp.tile([3, CB], F32, tag="ob", name="ob")

            for mj, (m0_, m1_) in enumerate(m3p):
                r = m1_ - m0_
                ks = blocks3[mj]
                for h in range(2):
                    hb = slice(h * 1024, (h + 1) * 1024)
                    ps = psp.tile([128, 1024], F32, tag="ps", name="ps")
                    for s in range(2):
                        sl = slice(h * 1024 + s * NSL, h * 1024 + (s + 1) * NSL)
                        pl = slice(s * NSL, (s + 1) * NSL)
                        for i, ki in enumerate(ks):
                            nc.tensor.matmul(ps[:r, pl], t3s[ki][:, m0_:m1_],
                                             pool[ki][:, sl],
                                             start=(i == 0), stop=(i == len(ks) - 1))
                    if mj < 4:
                        nc.scalar.activation(h3[mj][:, hb], ps[:r], AF.Relu,
                                             bias=b3[:r, mj:mj + 1])
                    else:
                        nc.vector.tensor_scalar(h3t4[0:64, hb], ps[:r],
                                                b3[:r, mj:mj + 1], 0.0,
                                                ALU.add, ALU.max)

            rhs5 = [h3[0], h3[1], h3[2], h3[3], h3t4]
            for mj, (m0_, m1_) in enumerate(((0, 128), (128, 192))):
                r = m1_ - m0_
                for h in range(2):
                    hb = slice(h * 1024, (h + 1) * 1024)
                    ps = psp.tile([128, 1024], F32, tag="ps", name="ps")
                    for s in range(2):
                        sl = slice(h * 1024 + s * NSL, h * 1024 + (s + 1) * NSL)
                        pl = slice(s * NSL, (s + 1) * NSL)
                        for i in range(5):
                            nc.tensor.matmul(ps[:r, pl], w1s[i][:, m0_:m1_],
                                             rhs5[i][:, sl],
                                             start=(i == 0), stop=(i == 4))
                    dst = amlp[:, hb] if mj == 0 else fmlp[64:128, hb]
                    nc.vector.tensor_scalar(dst, ps[:r], bm1[:r, mj:mj + 1], 0.0,
                                            ALU.add, ALU.max)

            for h in range(2):
                hb = slice(h * 1024, (h + 1) * 1024)
                ps = psp.tile([128, 1024], F32, tag="ps", name="ps")
                for s in range(2):
                    sl = slice(h * 1024 + s * NSL, h * 1024 + (s + 1) * NSL)
                    pl = slice(s * NSL, (s + 1) * NSL)
                    nc.tensor.matmul(ps[:64, pl], w2, amlp[:, sl],
                                     start=True, stop=True)
                nc.vector.tensor_scalar(fmlp[0:64, hb], ps[:64],
                                        bm2[:64, 0:1], 0.0, ALU.add, ALU.max)

            for h in range(2):
                hb = slice(h * 1024, (h + 1) * 1024)
                ph = psp.tile([128, 1024], F32, tag="ps", name="ph")
                for s in range(2):
                    sl = slice(h * 1024 + s * NSL, h * 1024 + (s + 1) * NSL)
                    pl = slice(s * NSL, (s + 1) * NSL)
                    nc.tensor.matmul(ph[:3, pl], w3, fmlp[:, sl],
                                     start=True, stop=True)
                nc.scalar.activation(lb[:, hb], ph[:3], AF.Identity,
                                     bias=bh[:, 0:1])
            for h in range(2):
                hb = slice(h * 1024, (h + 1) * 1024)
                pd = psp.tile([128, 1024], F32, tag="ps", name="pd")
                for s in range(2):
                    sl = slice(h * 1024 + s * NSL, h * 1024 + (s + 1) * NSL)
                    pl = slice(s * NSL, (s + 1) * NSL)
                    nc.tensor.matmul(pd[:3, pl], smat, lb[:, sl],
                                     start=True, stop=True)
                nc.scalar.activation(ob[:, hb], pd[:3], AF.Tanh,
                                     bias=zc[:3], scale=0.5)
            nc.vector.tensor_scalar(ob, ob, 0.5, 0.5, ALU.mult, ALU.add)
            nc.sync.dma_start(out=out_d[:, c0:c0 + CB], in_=ob)

        stage_a(0, None)
        qfb = emit_quantum()
        # chunk 0 quantum rows: stage_a(0) ran before qfb existed
        _, h3t4_0 = st[0]
        for g in range(4):
            nc.sync.dma_start(out=h3t4_0[64:72, g * 512:(g + 1) * 512],
                              in_=qfb[g * 8:(g + 1) * 8, :])
        stage_a(1, qfb)
        stage_b(0)
        stage_a(2, qfb)
        stage_b(1)
        stage_a(3, qfb)
        stage_b(2)
        stage_b(3)

    nc.compile()
    return nc


def _prep_host(inputs):
    conv1_w = np.asarray(inputs["conv1_w"], np.float32)
    conv2_w = np.asarray(inputs["conv2_w"], np.float32)
    conv3_w = np.asarray(inputs["conv3_w"], np.float32)
    T1, T2, T3 = _build_conv_maps(conv1_w, conv2_w, conv3_w)

    m2p_b = [(p * 288 + o0, p * 288 + o1)
             for p in range(4) for o0, o1 in ((0, 96), (96, 192), (192, 288))]
    _cache["blocks2"] = _nonzero_blocks(T2, _parts(576), m2p_b)
    _cache["blocks3"] = _nonzero_blocks(T3, _parts(288, 96), _parts(576))

    # MLP weights, conv rows permuted into my pos-major H3 ordering
    pt_w1 = np.asarray(inputs["pt_w1"], np.float32)
    cf_w1 = np.asarray(inputs["cf_w1"], np.float32)
    perm = np.empty(584, np.int64)
    for pos in range(9):
        for co in range(64):
            perm[pos * 64 + co] = co * 9 + pos
    perm[576:] = np.arange(576, 584)
    W1 = np.concatenate([pt_w1[perm], cf_w1[perm]], axis=1)  # [584, 192]

    W3 = np.zeros((128, 3), np.float32)
    W3[0:64, 0:2] = np.asarray(inputs["pt_w3"], np.float32)
    W3[64:128, 2] = np.asarray(inputs["cf_w2"], np.float32)[:, 0]

    S = np.zeros((3, 3), np.float32)
    S[:, 0] = (1, -1, 0)
    S[:, 1] = (-1, 1, 0)
    S[:, 2] = (0, 0, 1)

    def pack_bias(b, per, ntile, rows):
        full = np.tile(np.asarray(b, np.float32), rows // per * ntile)[:rows * ntile]
        out = np.zeros((128, ntile), np.float32)
        for m in range(ntile):
            seg = full[m * 128:(m + 1) * 128] if rows * ntile - m * 128 >= 128 \
                else np.pad(full[m * 128:], (0, 128 - (rows * ntile - m * 128)))
            out[:len(seg), m] = seg
        return out

    def pack_bias2(bvec, total, ntile):
        full = np.zeros(ntile * 128, np.float32)
        full[:total] = bvec
        return full.reshape(ntile, 128).T.copy()

    b1 = pack_bias2(np.tile(np.asarray(inputs["conv1_b"], np.float32), 36), 576, 5)
    b2f = np.tile(np.asarray(inputs["conv2_b"], np.float32), 36)
    m2p_host = [(p * 288 + o0, p * 288 + o1)
                for p in range(4) for o0, o1 in ((0, 96), (96, 192), (192, 288))]
    b2 = np.zeros((128, 12), np.float32)
    for jj, (s0, s1) in enumerate(m2p_host):
        b2[:s1 - s0, jj] = b2f[s0:s1]
    b3 = pack_bias2(np.tile(np.asarray(inputs["conv3_b"], np.float32), 9), 576, 5)
    bm1 = pack_bias2(np.concatenate([np.asarray(inputs["pt_b1"], np.float32),
                                     np.asarray(inputs["cf_b1"], np.float32)]), 192, 2)
    bm2 = pack_bias2(np.asarray(inputs["pt_b2"], np.float32), 64, 1)
    bh = np.concatenate([np.asarray(inputs["pt_b3"], np.float32),
                         np.asarray(inputs["cf_b2"], np.float32)]).reshape(3, 1)

    qp = np.asarray(inputs["quantum_params"], np.float32)  # [3,8,3]
    rot = np.zeros((128, 9), np.float32)
    for g in range(16):
        for q in range(8):
            for l in range(3):
                for i in range(3):
                    rot[q + 8 * g, l * 3 + i] = qp[l, q, i]

    wb16 = np.zeros((128, NC16), np.float32)
    def p16(name, arr):
        r, cc = arr.shape
        wb16[:r, OFF16[name]:OFF16[name] + cc] = arr
    p16("t1", T1)
    k2 = _parts(576); k3 = _parts(288, 96); km = _parts(584)
    for i, (k0, k1) in enumerate(k2): p16(f"t2_{i}", T2[k0:k1])
    for i, (k0, k1) in enumerate(k3): p16(f"t3_{i}", T3[k0:k1])
    for i, (k0, k1) in enumerate(km): p16(f"w1_{i}", W1[k0:k1])
    p16("w2", np.asarray(inputs["pt_w2"], np.float32))
    p16("w3", W3)
    p16("smat", S)

    wb32 = np.zeros((128, NC32), np.float32)
    def p32(name, arr):
        r, cc = arr.shape
        wb32[:r, OFF32[name]:OFF32[name] + cc] = arr
    p32("b1", b1); p32("b2", b2); p32("b3", b3)
    p32("bm1", bm1); p32("bm2", bm2); p32("bh", bh); p32("rot", rot)

    shared = {"wb16": wb16.astype(nbf)}

    board = np.asarray(inputs["board_state"], np.float32).reshape(B, 108)
    in_maps = []
    for c in range(NCORES):
        bx = board[c * BC:(c + 1) * BC]          # [8192, 108]
        xq = bx[:, :NQ]                           # [8192, 8]
        xqn = np.roll(xq, -1, axis=1)
        m = dict(shared)
        m["xT"] = np.ascontiguousarray(bx.T).astype(nbf)
        wb32c = wb32.copy()
        wb32c[:, OFF32["qx"]:OFF32["qx"] + BC // 16] = \
            xq.reshape(16, BC // 16, 8).transpose(0, 2, 1).reshape(128, BC // 16)
        wb32c[:, OFF32["qxn"]:OFF32["qxn"] + BC // 16] = \
            xqn.reshape(16, BC // 16, 8).transpose(0, 2, 1).reshape(128, BC // 16)
        m["wb32"] = wb32c
        in_maps.append(m)
    return in_maps


def kernel(**inputs):
    in_maps = _prep_host(inputs)
    if "nc" not in _cache:
        _cache["nc"] = _build_program()
    import os
    trace = os.environ.get("BASS_TRACE", "0") == "1"
    res = run_bass_kernel_spmd(_cache["nc"], in_maps, core_ids=list(range(NCORES)),
                               trace=trace)
    if res.exec_time_ns is not None:
        print(f"HW exec time: {res.exec_time_ns} ns")
        if res.instructions_and_trace is not None:
            print("trace:", res.instructions_and_trace[1])
    out = np.empty((B, 3), np.float32)
    for c in range(NCORES):
        out[c * BC:(c + 1) * BC] = res.results[c]["out"].T
    return out


if __name__ == "__main__":
    rng = np.random.default_rng(0)
    fake = {
        "board_state": rng.standard_normal((B, 3, 6, 6), dtype=np.float32),
        "target_positions": np.zeros((4, 2), np.int64),
        "conv1_w": rng.standard_normal((16, 3, 3, 3), dtype=np.float32) * 0.1,
        "conv1_b": rng.standard_normal(16, dtype=np.float32) * 0.1,
        "conv2_w": rng.standard_normal((32, 16, 3, 3), dtype=np.float32) * 0.05,
        "conv2_b": rng.standard_normal(32, dtype=np.float32) * 0.1,
        "conv3_w": rng.standard_normal((64, 32, 3, 3), dtype=np.float32) * 0.05,
        "conv3_b": rng.standard_normal(64, dtype=np.float32) * 0.1,
        "quantum_params": rng.standard_normal((3, 8, 3), dtype=np.float32),
        "pt_w1": rng.standard_normal((584, 128), dtype=np.float32) * 0.04,
        "pt_b1": rng.standard_normal(128, dtype=np.float32) * 0.04,
        "pt_w2": rng.standard_normal((128, 64), dtype=np.float32) * 0.09,
        "pt_b2": rng.standard_normal(64, dtype=np.float32) * 0.09,
        "pt_w3": rng.standard_normal((64, 2), dtype=np.float32) * 0.125,
        "pt_b3": rng.standard_normal(2, dtype=np.float32) * 0.125,
        "cf_w1": rng.standard_normal((584, 64), dtype=np.float32) * 0.04,
        "cf_b1": rng.standard_normal(64, dtype=np.float32) * 0.04,
        "cf_w2": rng.standard_normal((64, 1), dtype=np.float32) * 0.125,
        "cf_b2": rng.standard_normal(1, dtype=np.float32) * 0.125,
    }
    o = kernel(**fake)
    print(o.shape, o[:2])
